# revision 55
# baseline (speedup 1.0000x reference)
"""Causal self-attention on 8 Trainium2 NeuronCores (Bass/Tile).

Problem: B=2, S=2048, D=1024, H=16 heads (hd=64), fp32 in/out at the API.

Sharding (SPMD, same NEFF on 8 cores): core c -> batch b = c//4,
head-group g = c%4 (4 heads = 256 cols of wq/wk/wv, 256 rows of wo).
Host sums the 4 partial out-projections per batch and adds bo + bv@wo.

v3 vs v2 (157546ns -> 140920ns in the graded cost-model sim, HW-verified
rel err 4.45e-03):
  - head: per-ktile DMA batches (packed [wk|wq|wv] + x front halves) feed
    8 interleaved PE chains (kq for s-chunk 0 + v blocks 0-3) so the PE
    starts ~3.5us in and tracks the DMA stream.
  - everything else (kq s-chunks 1-3, v blocks 4-15, out-projections) is
    emitted as small filler units inside the attention pairs; the Tile
    scheduler (dependency-driven, freely reordering) does the placement.
  - PSUM: four static [P,2,QC] tiles = all 8 banks. A/B are the scores
    ping-pong; Y and F ALTERNATE per attention pair between AV-accumulator
    and filler-accumulator roles, so a pair's first AV never waits on the
    previous pair's evacuation (deps are tile-granular, not region-
    granular, which makes this alternation matter).
  - causal mask: in-place DVE multiply of pt's first 128 columns by a
    host-materialized [128, 2x128] 0/1 triangle after exp (a stride-0
    broadcast AP here is rejected by the BIR verifier; PE mask matmuls
    not needed).
  - softmax normalization: evacuate y (incl the ones-row sums) to bf16
    SBUF first (frees the PSUM banks fast), bf16 reciprocal on DVE,
    partition broadcasts on Pool, bf16 multiplies on DVE (4x mode), and
    the odd head's rows reach yT partitions 64:128 via a DVE
    stream_shuffle with offset partition bases (works on HW) instead of
    an SBUF DMA round-trip.
  - each pair's last two diagonal blocks (256+128 cols) share one
    scores bank and ONE exp instruction (the second matmul lands at
    column offset 256 with start=False: has_written is clear there, so
    it overwrites) — one fewer ACT instruction per pair.
  - tail: the last pair drains its fillers and starts the final
    out-projection's pair-0 matmuls before its normalization chain;
    reciprocal/multiplies read PSUM directly; out-projection etiles 6-7
    of q-chunk 2 are held back past the last pair (evacuating on the
    tail-idle ScalarE) to fill the normalization latency window; out
    DMAs go in 2-etile chunks.
  (Dual-PSUM-operand tensor_mul and K=1 PE broadcasts fail neuronxcc
  compilation; fp8/DoubleRow rejected on numerics.)
"""

import numpy as np
import ml_dtypes

import concourse.bass as bass
import concourse.tile as tile
from concourse import bacc, mybir
from concourse.bass_utils import run_bass_kernel_spmd

P = 128
B, S, D, H, HD = 2, 2048, 1024, 16, 64
JH = 256          # head-dim columns per core (4 heads x 64)
KT = D // P       # 8 contraction tiles for the projections
QC = 512          # query-chunk (matmul moving free dim)
NQC = S // QC     # 4
NKB = S // P      # 16 key blocks
F32 = mybir.dt.float32
BF16 = mybir.dt.bfloat16
FP16 = mybir.dt.float16
AF = mybir.ActivationFunctionType
BF16NP = ml_dtypes.bfloat16

IDENT32 = list(range(32))

LABELS = {}
FOLLOW = None  # optional predicate(label) -> bool for tile_follow debugging

_NC = None


class FillerQueue:
    """Callable units of deferred PE work, popped inside attention pairs."""

    def __init__(self):
        self.q = []

    def add(self, units):
        self.q.extend(units)

    def pop(self, n=1):
        for _ in range(n):
            if self.q:
                self.q.pop(0)()

    def drain(self):
        while self.q:
            self.q.pop(0)()


def build(repeats: int = 1, num_devices: int = 8, debug_outs: bool = False):
    nc = bacc.Bacc(
        "TRN2", target_bir_lowering=False, debug=False, num_devices=num_devices
    )

    xT_d = nc.dram_tensor("xT", [D, S], BF16, kind="ExternalInput").ap()
    wkqv_d = nc.dram_tensor("wkqv", [D, 3 * JH], BF16, kind="ExternalInput").ap()
    wo_d = nc.dram_tensor("wo", [JH, D], BF16, kind="ExternalInput").ap()
    bq_d = nc.dram_tensor("bq", [JH], F32, kind="ExternalInput").ap()
    bk_d = nc.dram_tensor("bk", [JH], F32, kind="ExternalInput").ap()
    tri_d = nc.dram_tensor("tric", [P, 2 * P], BF16, kind="ExternalInput").ap()
    vones_d = nc.dram_tensor("vones", [P, 1], BF16, kind="ExternalInput").ap()
    outT_d = nc.dram_tensor("outT", [D, S], FP16, kind="ExternalOutput").ap()

    xT_re = xT_d.rearrange("(o p) s -> p o s", p=P)        # [128, 8, 2048]
    wkqv_re = wkqv_d.rearrange("(o p) j -> p o j", p=P)    # [128, 8, 768]
    wo_re = wo_d.rearrange("(o p) e -> p o e", p=P)        # [128, 2, 1024]
    bq_re = bq_d.rearrange("(t p) -> p t", p=P)            # [128, 2]
    bk_re = bk_d.rearrange("(t p) -> p t", p=P)
    outT_re = outT_d.rearrange("(o p) s -> p o s", p=P)    # [128, 8, 2048]

    LABELS.clear()

    def lab(inst, s):
        try:
            LABELS[inst.ins.name] = s
        except Exception:
            try:
                LABELS[inst.name] = s
            except Exception:
                pass
        if FOLLOW is not None and FOLLOW(s):
            tile.tile_follow(inst, log_all_deps=True)
        return inst

    with tile.TileContext(nc) as tc:
        with tc.tile_pool(name="persist", bufs=1) as persist:
            xT = persist.tile([P, KT, S], BF16, tag="xT")
            # packed weights: [:, kt, 0:256]=wk, [256:512]=wq, [512:768]=wv
            wsb = persist.tile([P, KT, 3 * JH], BF16, tag="wkqv")
            wo_sb = persist.tile([P, 2, D], BF16, tag="wo")
            bq_sb = persist.tile([P, 2], F32, tag="bq")
            bk_sb = persist.tile([P, 2], F32, tag="bk")
            tri_sb = persist.tile([P, 2, P], BF16, tag="tri")
            vones_sb = persist.tile([P, 1], BF16, tag="vones")
            qT = persist.tile([P, 2, S], BF16, tag="qT")
            kT = persist.tile([P, 2, S], BF16, tag="kT")
            # per (k-block, head): [v(0:64) | ones(64)]
            vsb = persist.tile([P, NKB, 4, 65], BF16, tag="vsb")
            yT = persist.tile([P, 2, S], BF16, tag="yT")

            def wk_sl(kt, jt):
                return wsb[:, kt, jt * P : (jt + 1) * P]

            def wq_sl(kt, jt):
                return wsb[:, kt, JH + jt * P : JH + (jt + 1) * P]

            def wv_sl(kt):
                return wsb[:, kt, 2 * JH : 3 * JH]

            # ---- DMA: per-ktile batches sized to the PE's head consumption
            # (one [128,768] weight chunk + one [128,1024] x front half per
            # ktile ~= 1.27us of DMA vs ~1.28us of PE work per ktile) ----
            for kt in range(KT):
                nc.sync.dma_start(wsb[:, kt, :], wkqv_re[:, kt, :])
                if kt == 0:
                    # split the first x chunk so the PE starts sooner
                    nc.sync.dma_start(xT[:, 0, 0:QC], xT_re[:, 0, 0:QC])
                    nc.sync.dma_start(xT[:, 0, QC:2 * QC], xT_re[:, 0, QC:2 * QC])
                else:
                    nc.sync.dma_start(
                        xT[:, kt, 0:2 * QC], xT_re[:, kt, 0:2 * QC]
                    )
            nc.sync.dma_start(bk_sb[:], bk_re)
            nc.sync.dma_start(bq_sb[:], bq_re)
            nc.sync.dma_start(vones_sb[:], vones_d)
            nc.sync.dma_start(tri_sb[:], tri_d)
            for kt in range(KT):
                nc.sync.dma_start(
                    xT[:, kt, 2 * QC : S], xT_re[:, kt, 2 * QC : S]
                )
            nc.sync.dma_start(wo_sb[:], wo_re)

            # PSUM: four static [P,2,QC] fp32 tiles = all 8 banks.
            psum = tc.alloc_tile_pool(name="psum", bufs=1, space="PSUM")
            A = psum.tile([P, 2, QC], F32, tag="A")   # scores ping
            Bt = psum.tile([P, 2, QC], F32, tag="B")  # scores pong
            Y = psum.tile([P, 2, QC], F32, tag="Y")   # AV accumulators
            Ft = psum.tile([P, 2, QC], F32, tag="F")  # filler accumulators

            pt_pool = tc.alloc_tile_pool(name="pt", bufs=4)
            u_pool = tc.alloc_tile_pool(name="u", bufs=2)
            rt_pool = tc.alloc_tile_pool(name="rt", bufs=2)
            rb_pool = tc.alloc_tile_pool(name="rb", bufs=2)
            yn_pool = tc.alloc_tile_pool(name="yn", bufs=2)
            ostage = tc.alloc_tile_pool(name="ostage", bufs=4)

            # ---- head: kq(sc0) on A/B + v(blocks 0-3) on Y/F, ktile-major
            # tracking the DMA stream ----
            for kt in range(KT):
                st, sp = kt == 0, kt == KT - 1
                for jt in range(2):  # k chains
                    lab(nc.tensor.matmul(
                        A[:, jt, :], wk_sl(kt, jt), xT[:, kt, 0:QC],
                        start=st, stop=sp,
                    ), f"head.k kt{kt} jt{jt}")
                if sp:
                    # k evacuations right after the last k matmuls so the
                    # first scores' stationary is ready when the PE gets there
                    for jt in range(2):
                        lab(nc.scalar.activation(
                            kT[:, jt, 0:QC], A[:, jt, :], AF.Identity,
                            bias=bk_sb[:, jt : jt + 1],
                        ), f"head.kev jt{jt}")
                for blk in range(4):  # v chains, blocks 0-3
                    t2 = (Y, Ft)[blk // 2]
                    lab(nc.tensor.matmul(
                        t2[:, blk % 2, 0:JH],
                        xT[:, kt, blk * P : (blk + 1) * P], wv_sl(kt),
                        start=st, stop=sp,
                    ), f"head.v kt{kt} b{blk}")
                if sp:
                    for blk in range(4):
                        t2 = (Y, Ft)[blk // 2]
                        src = t2[:, blk % 2, 0:JH].rearrange(
                            "p (h j) -> p h j", h=4
                        )
                        if blk % 2 == 0:
                            nc.scalar.copy(vsb[:, blk, :, 0:64], src)
                        else:
                            nc.vector.tensor_copy(vsb[:, blk, :, 0:64], src)
                for jt in range(2):  # q chains
                    lab(nc.tensor.matmul(
                        Bt[:, jt, :], wq_sl(kt, jt), xT[:, kt, 0:QC],
                        start=st, stop=sp,
                    ), f"head.q kt{kt} jt{jt}")
                if sp:
                    for jt in range(2):
                        lab(nc.vector.tensor_scalar_add(
                            qT[:, jt, 0:QC], Bt[:, jt, :], bq_sb[:, jt : jt + 1]
                        ), f"head.qev jt{jt}")
            # ones column of every [v|ones] stationary
            nc.vector.tensor_copy(
                vsb[:, :, :, 64:65],
                vones_sb[:, None, None, :].to_broadcast((P, NKB, 4, 1)),
            )

            # ---- filler unit generators ----
            def kq_units(sc, t, FT):
                """One kq pass (t=0: k, t=1: q) for s-chunk sc on FT."""
                w_sl = (wk_sl, wq_sl)[t]
                Ft_ = FT
                units = []
                for kt in range(KT):
                    def u(kt=kt):
                        for jt in range(2):
                            lab(nc.tensor.matmul(
                                Ft_[:, jt, :], w_sl(kt, jt),
                                xT[:, kt, sc * QC : (sc + 1) * QC],
                                start=(kt == 0), stop=(kt == KT - 1),
                            ), f"f.kq sc{sc} t{t} kt{kt}")
                    units.append(u)

                def evac(t=t, sc=sc):
                    for jt in range(2):
                        if t == 0:
                            lab(nc.scalar.activation(
                                kT[:, jt, sc * QC : (sc + 1) * QC],
                                Ft_[:, jt, :], AF.Identity,
                                bias=bk_sb[:, jt : jt + 1],
                            ), f"f.kqev sc{sc} t{t}")
                        else:
                            lab(nc.vector.tensor_scalar_add(
                                qT[:, jt, sc * QC : (sc + 1) * QC],
                                Ft_[:, jt, :], bq_sb[:, jt : jt + 1],
                            ), f"f.kqev sc{sc} t{t}")
                units.append(evac)
                return units

            def v_units(blk0, FT):
                """v chains for blocks blk0, blk0+1 on FT."""
                Ft_ = FT
                units = []
                for kt in range(KT):
                    def u(kt=kt):
                        for i in range(2):
                            blk = blk0 + i
                            lab(nc.tensor.matmul(
                                Ft_[:, i, 0:JH],
                                xT[:, kt, blk * P : (blk + 1) * P], wv_sl(kt),
                                start=(kt == 0), stop=(kt == KT - 1),
                            ), f"f.v b{blk} kt{kt}")
                    units.append(u)

                def evac():
                    for i in range(2):
                        blk = blk0 + i
                        src = Ft_[:, i, 0:JH].rearrange("p (h j) -> p h j", h=4)
                        if i == 0:
                            lab(nc.scalar.copy(vsb[:, blk, :, 0:64], src),
                                f"f.vev b{blk}")
                        else:
                            lab(nc.vector.tensor_copy(vsb[:, blk, :, 0:64], src),
                                f"f.vev b{blk}")
                units.append(evac)
                return units

            def outproj_units(qc, e0, FT, act_ev=False):
                """Out-projection for etiles e0, e0+1 of q-chunk qc on FT;
                stages fp16 and DMAs the 2-etile chunk out when done."""
                Ft_ = FT
                ot = ostage.tile([P, 2, QC], FP16, tag="ot")
                sl = slice(qc * QC, (qc + 1) * QC)
                units = []
                for i in range(2):
                    def mm(i=i):
                        et = e0 + i
                        for pair in range(2):
                            lab(nc.tensor.matmul(
                                Ft_[:, i, :],
                                wo_sb[:, pair, et * P : (et + 1) * P],
                                yT[:, pair, sl],
                                start=(pair == 0), stop=(pair == 1),
                            ), f"f.op qc{qc} et{et} p{pair}")
                    def ev(i=i):
                        if act_ev:
                            lab(nc.scalar.copy(ot[:, i, :], Ft_[:, i, :]),
                                f"f.opev qc{qc} et{e0 + i}")
                        else:
                            lab(nc.vector.tensor_copy(ot[:, i, :], Ft_[:, i, :]),
                                f"f.opev qc{qc} et{e0 + i}")
                        if i == 1:
                            lab(nc.sync.dma_start(
                                outT_re[:, e0 : e0 + 2, sl], ot[:]
                            ), f"f.opdma qc{qc} e{e0}")
                    units.append(mm)
                    units.append(ev)
                return units

            # ---- attention ----
            ACT_NS_PER_EL = 0.833
            PE_NS_PER_ROW = 0.4167

            def attn_pair(qc, pair, fq, YA, last=False):
                nkb = 4 * (qc + 1)
                sl = slice(qc * QC, (qc + 1) * QC)
                pts = {}

                def scores_exp(kb, sc_t, merge_next=False):
                    """Scores+exp for block kb; with merge_next, the next
                    (smaller) diagonal block shares the same PSUM tile and
                    exp instruction — its columns land at tile offset n_d
                    via has_written overwrite semantics."""
                    d = kb - 4 * qc
                    n_d = QC - 128 * d if d > 0 else QC
                    q_off = qc * QC + (QC - n_d)
                    n2 = QC - 128 * (d + 1) if merge_next else 0
                    for he in range(2):
                        lab(nc.tensor.matmul(
                            sc_t[:, he, 0:n_d],
                            kT[64 * he : 64 * he + 64, pair,
                               kb * P : (kb + 1) * P],
                            qT[64 * he : 64 * he + 64, pair,
                               q_off : q_off + n_d],
                            start=True, stop=not merge_next,
                            tile_position=(64 * he, 0),
                        ), f"sc q{qc}p{pair} kb{kb} he{he}")
                    if merge_next:
                        q2 = qc * QC + (QC - n2)
                        for he in range(2):
                            lab(nc.tensor.matmul(
                                sc_t[:, he, n_d : n_d + n2],
                                kT[64 * he : 64 * he + 64, pair,
                                   (kb + 1) * P : (kb + 2) * P],
                                qT[64 * he : 64 * he + 64, pair,
                                   q2 : q2 + n2],
                                start=False, stop=True,
                                tile_position=(64 * he, 0),
                            ), f"sc q{qc}p{pair} kb{kb + 1} he{he}")
                    pt = pt_pool.tile([P, 2, QC], BF16, tag="pt")
                    lab(nc.scalar.activation(
                        pt[:, :, 0:n_d + n2], sc_t[:, :, 0:n_d + n2],
                        AF.Exp, scale=0.125
                    ), f"exp q{qc}p{pair} kb{kb}")
                    if d >= 0:
                        # causal mask: zero the upper triangle of the first
                        # 128 columns (the only partially-masked region)
                        lab(nc.vector.tensor_mul(
                            pt[:, :, 0:P], pt[:, :, 0:P], tri_sb[:, :, :],
                        ), f"mask q{qc}p{pair} kb{kb}")
                    if merge_next:
                        lab(nc.vector.tensor_mul(
                            pt[:, :, n_d : n_d + P],
                            pt[:, :, n_d : n_d + P], tri_sb[:, :, :],
                        ), f"mask q{qc}p{pair} kb{kb + 1}")
                        pts[kb + 1] = (pt, n2, n_d)
                    pts[kb] = (pt, n_d, 0)

                scores_exp(0, A)
                fq.pop(3)
                act_t, pe_t = 0.0, 0.0
                for kb in range(nkb):
                    if kb + 1 == nkb - 2:
                        # last two (smallest) diagonal blocks share one
                        # scores tile and one exp instruction
                        scores_exp(kb + 1, (A, Bt)[(kb + 1) % 2],
                                   merge_next=True)
                    elif kb + 1 < nkb - 2:
                        scores_exp(kb + 1, (A, Bt)[(kb + 1) % 2])
                    pt, n_d, off = pts.pop(kb)
                    h0 = 2 * pair
                    for he in range(2):
                        lab(nc.tensor.matmul(
                            YA[0:65, he, QC - n_d : QC],
                            vsb[:, kb, h0 + he, :],
                            pt[:, he, off : off + n_d],
                            start=(kb == 0), stop=(kb == nkb - 1),
                        ), f"av q{qc}p{pair} kb{kb} he{he}")
                    act_t += 2 * n_d * ACT_NS_PER_EL + 290
                    pe_t += 4 * n_d * PE_NS_PER_ROW
                    fq.pop(1)
                    pe_t += 420
                    while act_t > pe_t + 300 and fq.q:
                        fq.q.pop(0)()
                        pe_t += 420
                if last:
                    fq.drain()
                # ---- normalization ----
                yn = yn_pool.tile([64, QC], BF16, tag="yn")
                with nc.allow_low_precision(
                    reason="softmax normalization in reduced precision"
                ):
                    if last:
                        # tail-latency-optimized: drain fillers + start the
                        # final out-projection's pair-0 matmuls first (they
                        # only need yT(3,0)), reciprocals straight from PSUM,
                        # partition-broadcast on the otherwise-idle PE (K=1
                        # matmul against a ones row of tri) into Bt's banks,
                        # multiplies straight from PSUM.
                        tsl = slice(3 * QC, 4 * QC)
                        other = Ft if YA is Y else Y
                        for gi, t2 in ((0, A), (1, other)):
                            for i in range(2):
                                et = 2 * gi + i
                                lab(nc.tensor.matmul(
                                    t2[:, i, :],
                                    wo_sb[:, 0, et * P : (et + 1) * P],
                                    yT[:, 0, tsl],
                                    start=True, stop=False,
                                ), f"tail.p0 et{et}")
                        rt = rt_pool.tile([1, 2, QC], BF16, tag="rt")
                        lab(nc.vector.reciprocal(rt[0:1, :, :], YA[64:65, :, :]),
                            "tail.recip")
                        rbf = rb_pool.tile([64, 2, QC], BF16, tag="rbf")
                        lab(nc.gpsimd.partition_broadcast(
                            rbf[0:64, 1, :], rt[0:1, 1, :]), "tail.bc1")
                        lab(nc.gpsimd.partition_broadcast(
                            rbf[0:64, 0, :], rt[0:1, 0, :]), "tail.bc0")
                        lab(nc.vector.tensor_mul(
                            yn[0:64, :], YA[0:64, 1, :], rbf[0:64, 1, :]
                        ), "tail.mul1")
                        lab(nc.vector.tensor_mul(
                            yT[0:64, pair, sl], YA[0:64, 0, :], rbf[0:64, 0, :]
                        ), "tail.mul0")
                    else:
                        # evacuate y (incl sums rows) to bf16 first: frees
                        # the PSUM banks fast, everything else reads SBUF
                        u = u_pool.tile([65, 2, QC], BF16, tag="u")
                        lab(nc.vector.tensor_copy(u[:, 0, :], YA[0:65, 0, :]),
                            f"n.ev0 q{qc}p{pair}")
                        lab(nc.vector.tensor_copy(u[:, 1, :], YA[0:65, 1, :]),
                            f"n.ev1 q{qc}p{pair}")
                        fq.pop(2)
                        rtb = rt_pool.tile([1, 2, QC], BF16, tag="rtb")
                        lab(nc.vector.reciprocal(rtb[0:1, :, :], u[64:65, :, :]),
                            f"n.recip q{qc}p{pair}")
                        rbb = rb_pool.tile([64, 2, QC], BF16, tag="rbb")
                        lab(nc.gpsimd.partition_broadcast(
                            rbb[0:64, 0, :], rtb[0:1, 0, :]
                        ), f"n.bc0 q{qc}p{pair}")
                        lab(nc.gpsimd.partition_broadcast(
                            rbb[0:64, 1, :], rtb[0:1, 1, :]
                        ), f"n.bc1 q{qc}p{pair}")
                        fq.pop(2)
                        lab(nc.vector.tensor_mul(
                            yT[0:64, pair, sl], u[0:64, 0, :], rbb[0:64, 0, :]
                        ), f"n.mul0 q{qc}p{pair}")
                        lab(nc.vector.tensor_mul(
                            yn[0:64, :], u[0:64, 1, :], rbb[0:64, 1, :]
                        ), f"n.mul1 q{qc}p{pair}")
                # odd head's rows into yT partitions 64:128 (lane shift)
                lab(nc.vector.stream_shuffle(
                    yT[64:128, pair, sl], yn[0:64, :], mask=IDENT32
                ), f"n.shuf q{qc}p{pair}")
                fq.drain()

            def outproj_tail(y_last):
                """Final q-chunk out-projection: pair-0 halves for A/other
                were emitted inside the last pair's normalization; y_last and
                Bt groups here, then pair-1 + evacuations + chunked DMAs."""
                sl = slice(3 * QC, 4 * QC)
                other = Ft if y_last is Y else Y
                groups = ((0, A), (1, other), (3, Bt), (2, y_last))
                for gi, t2 in groups[2:]:
                    for i in range(2):
                        et = 2 * gi + i
                        lab(nc.tensor.matmul(
                            t2[:, i, :],
                            wo_sb[:, 0, et * P : (et + 1) * P],
                            yT[:, 0, sl],
                            start=True, stop=False,
                        ), f"tail.p0 et{et}")
                for gi, t2 in groups:
                    ot = ostage.tile([P, 2, QC], FP16, tag="ot")
                    for i in range(2):
                        et = 2 * gi + i
                        lab(nc.tensor.matmul(
                            t2[:, i, :],
                            wo_sb[:, 1, et * P : (et + 1) * P],
                            yT[:, 1, sl],
                            start=False, stop=True,
                        ), f"tail.p1 et{et}")
                        if i == 0:
                            lab(nc.scalar.copy(ot[:, i, :], t2[:, i, :]),
                                f"tail.ev et{et}")
                        else:
                            lab(nc.vector.tensor_copy(ot[:, i, :], t2[:, i, :]),
                                f"tail.ev et{et}")
                    lab(nc.sync.dma_start(
                        outT_re[:, 2 * gi : 2 * gi + 2, sl], ot[:]
                    ), f"tail.dma g{gi}")

            for _rep in range(repeats):
                fq = FillerQueue()
                ys = [(Y, Ft)[i % 2] for i in range(8)]
                fs = [(Y, Ft)[(i + 1) % 2] for i in range(8)]
                fq.add(kq_units(1, 0, fs[0])); fq.add(kq_units(1, 1, fs[0]))
                attn_pair(0, 0, fq, ys[0])
                fq.add(v_units(4, fs[1])); fq.add(v_units(6, fs[1]))
                attn_pair(0, 1, fq, ys[1])
                fq.add(kq_units(2, 0, fs[2])); fq.add(v_units(8, fs[2]))
                attn_pair(1, 0, fq, ys[2])
                fq.add(kq_units(2, 1, fs[3])); fq.add(v_units(10, fs[3]))
                attn_pair(1, 1, fq, ys[3])
                fq.add(kq_units(3, 0, fs[4])); fq.add(outproj_units(0, 0, fs[4]))
                fq.add(outproj_units(0, 2, fs[4])); fq.add(outproj_units(0, 4, fs[4]))
                fq.add(outproj_units(0, 6, fs[4]))
                attn_pair(2, 0, fq, ys[4])
                fq.add(kq_units(3, 1, fs[5])); fq.add(v_units(12, fs[5]))
                fq.add(v_units(14, fs[5]))
                attn_pair(2, 1, fq, ys[5])
                for e0 in (0, 2, 4, 6):
                    fq.add(outproj_units(1, e0, fs[6]))
                attn_pair(3, 0, fq, ys[6])
                fq.add(outproj_units(2, 0, fs[7]))
                fq.add(outproj_units(2, 2, fs[7]))
                fq.add(outproj_units(2, 4, fs[7]))
                late = outproj_units(2, 6, Bt, act_ev=True)
                attn_pair(3, 1, fq, ys[7], last=True)
                for u in late:
                    u()
                outproj_tail(ys[7])

            for pool in (ostage, yn_pool, rb_pool, rt_pool, u_pool, pt_pool,
                         psum):
                pool.release()

    nc.compile()
    return nc


def make_in_maps(x, wq, bq, wk, bk, wv, wo):
    """Per-core input dicts (8 cores): core c -> batch c//4, head-group c%4."""
    tri1 = np.where(
        np.arange(P)[None, :] >= np.arange(P)[:, None], 1.0, 0.0
    ).astype(BF16NP)
    tric = np.concatenate([tri1, tri1], axis=1)
    vones = np.ones((P, 1), dtype=BF16NP)
    in_maps = []
    for c in range(8):
        b, g = divmod(c, 4)
        jsl = slice(JH * g, JH * (g + 1))
        wkqv = np.concatenate(
            [wk[:, jsl], wq[:, jsl], wv[:, jsl]], axis=1
        )
        in_maps.append(
            {
                "xT": np.ascontiguousarray(x[b].T).astype(BF16NP),
                "wkqv": np.ascontiguousarray(wkqv).astype(BF16NP),
                "wo": np.ascontiguousarray(wo[jsl, :]).astype(BF16NP),
                "bq": np.ascontiguousarray(bq[jsl]).astype(np.float32),
                "bk": np.ascontiguousarray(bk[jsl]).astype(np.float32),
                "tric": tric,
                "vones": vones,
            }
        )
    return in_maps


def _get_nc():
    global _NC
    if _NC is None:
        _NC = build()
    return _NC


def kernel(x, wq, bq, wk, bk, wv, bv, wo, bo, **run_kwargs):
    x = np.asarray(x, dtype=np.float32)
    wq = np.asarray(wq, dtype=np.float32)
    bq = np.asarray(bq, dtype=np.float32)
    wk = np.asarray(wk, dtype=np.float32)
    bk = np.asarray(bk, dtype=np.float32)
    wv = np.asarray(wv, dtype=np.float32)
    bv = np.asarray(bv, dtype=np.float32)
    wo = np.asarray(wo, dtype=np.float32)
    bo = np.asarray(bo, dtype=np.float32)

    nc = _get_nc()
    in_maps = make_in_maps(x, wq, bq, wk, bk, wv, wo)
    res = run_bass_kernel_spmd(nc, in_maps, core_ids=list(range(8)), **run_kwargs)
    outs = [np.asarray(r["outT"]) for r in res.results]
    # softmax rows sum to 1, so the +bv contribution to out is exactly bv@wo
    bias = bo + bv @ wo
    y = np.empty((B, S, D), dtype=np.float32)
    for b in range(B):
        acc = (
            outs[4 * b].astype(np.float32)
            + outs[4 * b + 1].astype(np.float32)
            + outs[4 * b + 2].astype(np.float32)
            + outs[4 * b + 3].astype(np.float32)
        )
        y[b] = acc.T + bias[None, :]
    if run_kwargs:
        kernel.last_result = res
    return y
